# revision 4
# baseline (speedup 1.0000x reference)
"""DifferentiableTokenSelection Trainium2 kernel (v2: W-stationary mm1).

Math (reference):
    x: [b=2, t=64, n=1024, e=512] -> x_flat [b, m=65536, e]
    scores  = x_flat @ W.T + bias            [b, m, k=256]
    weights = softmax(scores / tau, axis=m)  (tau = 1.0)
    out     = einsum('bmk,bme->bke', weights, x_flat)   [b, 256, 512]

Key simplifications (exact, not approximations):
  * softmax over m is invariant to per-(b,k) constant shifts -> the bias
    cancels entirely; ignore b_bias.
  * scores ~ N(0,1), max |s| ~ 6 -> exp() without max-subtraction is safe.
    Single streaming pass: U[k,e] = sum_m exp(s[m,k]) x[m,e], denom[k] =
    sum_m exp(s[m,k]); out = U / denom, both from the SAME quantized
    exp-weights so quantization largely cancels in the ratio.

v2 pipeline (vs v1 which used x as the mm1 stationary operand and was
LDWEIGHTS-bound):
  * mm1 computes scoresT[k, m] with the 8 W.T tiles [128e, 128k] bf16 as
    the STATIONARY operand (cycled per block, loads hide under streaming)
    and host-pre-transposed x^T bf16 as the moving operand. Pure
    streaming: 8 matmuls x N=512 per 512-token block.
  * scalar exp reads the scoresT PSUM directly -> fp16 wexpT in SBUF;
    accum_out gives per-block denom partials for free.
  * DMA xbar transpose (16x128 tiles, 2-byte dtype) flips wexpT [k,m] ->
    wexp [m,k] fp16; a DVE copy casts to fp8 for mm2's lhsT.
  * mm2 (pooling) unchanged: fp8e4m3 DoubleRow, rhs = natural x fp8 pair
    [128,2,512], lhsT = wexp pair [128,2,128]; fp32 PSUM accumulation.
  * mm2 is emitted LAG blocks behind mm1 so the in-order PE queue never
    waits on the exp->transpose->cast chain.

Sharding: batch x token-axis. core i handles batch i//4, m-rows
[16384*(i%4), 16384*(i%4+1)). Each core emits partial U and denom; the
host sums the 4 partials per batch and divides (gather/unshard step).
"""

import numpy as np
import ml_dtypes

import concourse.bacc as bacc
import concourse.bass as bass
import concourse.tile as tile
from concourse import mybir
from concourse.bass_utils import run_bass_kernel_spmd

B, T, NTOK, E, K = 2, 64, 1024, 512, 256
M = T * NTOK                 # 65536 tokens per batch
NCORES = 8
CORES_PER_B = NCORES // B    # 4
RPC = M // CORES_PER_B       # 16384 rows per core
MBLK = 512                   # tokens per pipeline block
NBLK = RPC // MBLK           # 32
LAG = 2                      # mm2 trails mm1 by this many blocks

F32 = mybir.dt.float32
F16 = mybir.dt.float16
BF16 = mybir.dt.bfloat16
FP8 = mybir.dt.float8e4
EXP = mybir.ActivationFunctionType.Exp
BF = ml_dtypes.bfloat16
F8 = ml_dtypes.float8_e4m3
DR = mybir.MatmulPerfMode.DoubleRow


def build_nc(
    rows: int,
    xt_bufs: int = 3,
    xin_bufs: int = 4,
) -> bass.Bass:
    """Emit the per-core bass program for `rows` m-rows."""
    nblk = rows // MBLK
    assert rows % MBLK == 0

    nc = bacc.Bacc("TRN2", target_bir_lowering=False, debug=False)
    # mm1 moving operand: host-pre-transposed bf16 x^T.
    # xt[p, blk, ec, mm] = x[blk*512 + mm, ec*128 + p]
    xt_d = nc.dram_tensor("xt", [128, nblk, 4, MBLK], BF16, kind="ExternalInput")
    # mm2 rhs: natural x in fp8, SBUF-layout-matched.
    # x8[p, blk, j, e] = x[blk*512 + j*128 + p, e]
    x8_d = nc.dram_tensor("x8", [128, nblk, 4, E], FP8, kind="ExternalInput")
    # mm1 stationary tiles: wt[p, (ec*2+kc)*128 + i] = W.T[ec*128+p, kc*128+i]
    wt_d = nc.dram_tensor("wt", [128, 8 * 128], BF16, kind="ExternalInput")
    u_d = nc.dram_tensor("u", [2, 128, E], F32, kind="ExternalOutput")
    d_d = nc.dram_tensor("d", [128, 2, nblk], F32, kind="ExternalOutput")

    with tile.TileContext(nc) as tc:
        with (
            tc.tile_pool(name="const", bufs=1) as constp,
            tc.tile_pool(name="xt", bufs=xt_bufs) as xtp,
            tc.tile_pool(name="xin", bufs=xin_bufs) as xinp,
            tc.tile_pool(name="wexpT", bufs=3) as wexpTp,
            tc.tile_pool(name="wnat", bufs=3) as wnatp,
            tc.tile_pool(name="w8", bufs=LAG + 2) as w8p,
            tc.tile_pool(name="outs", bufs=1) as outp,
            tc.tile_pool(name="ps_sc", bufs=2, space="PSUM") as ps_sc,
            tc.tile_pool(name="ps_acc", bufs=1, space="PSUM") as ps_acc,
        ):
            wts = constp.tile([128, 8 * 128], BF16)
            nc.sync.dma_start(out=wts[:], in_=wt_d.ap())
            nexp_bias = constp.tile([128, 1], F32)
            nc.gpsimd.memset(nexp_bias[:], -2.7725887)  # -ln(16)
            den_parts = constp.tile([128, 2, nblk], F32)

            u_ps = ps_acc.tile([128, 2, E], F32)  # 2 banks, live all kernel

            def emit_front(blk):
                """loads + mm1 + exp + transpose + cast for block blk."""
                xtb = xtp.tile([128, 4, MBLK], BF16, tag="xtb")
                nc.sync.dma_start(out=xtb[:], in_=xt_d.ap()[:, blk])
                x8b = xinp.tile([128, 4, E], FP8, tag="x8b")
                nc.gpsimd.dma_start(out=x8b[:], in_=x8_d.ap()[:, blk])

                # mm1: scoresT[kc][128, m] += W.T-tile^T @ x^T-chunk
                sc = ps_sc.tile([128, 2, MBLK], F32, tag="sc")
                for kc in range(2):
                    for ec in range(4):
                        nc.tensor.matmul(
                            sc[:, kc, :],
                            wts[:, (ec * 2 + kc) * 128 : (ec * 2 + kc + 1) * 128],
                            xtb[:, ec, :],
                            start=(ec == 0),
                            stop=(ec == 3),
                            skip_group_check=True,
                        )
                # exp(s - ln16): keeps weights in fp8e4m3 range (max ~240;
                # raw exp can reach ~270). The 1/16 scale hits numerator and
                # denominator alike -> exact cancel. accum_out = denom part.
                wexpT = wexpTp.tile([128, 2, MBLK], F16, tag="wexpT")
                for kc in range(2):
                    nc.scalar.activation(
                        wexpT[:, kc, :],
                        sc[:, kc, :],
                        EXP,
                        bias=nexp_bias[:],
                        accum_out=den_parts[:, kc, blk : blk + 1],
                    )
                # DMA xbar transpose: wexpT [k, m] -> wnat [m-part, ms, k]
                wnat = wnatp.tile([128, 4, K], F16, tag="wnat")
                for kc in range(2):
                    for ms in range(4):
                        nc.scalar.dma_start_transpose(
                            wnat[:, ms, kc * 128 : (kc + 1) * 128],
                            wexpT[:, kc, ms * 128 : (ms + 1) * 128],
                        )
                w8 = w8p.tile([128, 4, K], FP8, tag="w8")
                nc.vector.tensor_copy(w8[:], wnat[:])
                return x8b, w8

            def emit_mm2(blk, x8b, w8):
                for jp in range(2):
                    first = blk == 0 and jp == 0
                    last = blk == nblk - 1 and jp == 1
                    for c in range(2):
                        nc.tensor.matmul(
                            u_ps[:, c, :],
                            w8[:, 2 * jp : 2 * jp + 2, c * 128 : (c + 1) * 128],
                            x8b[:, 2 * jp : 2 * jp + 2, :],
                            start=first,
                            stop=last,
                            perf_mode=DR,
                        )

            pending = []
            for blk in range(nblk):
                pending.append((blk, *emit_front(blk)))
                if blk >= LAG:
                    emit_mm2(*pending.pop(0))
            for item in pending:
                emit_mm2(*item)

            u_sb = outp.tile([128, 2, E], F32)
            nc.vector.tensor_copy(u_sb[:], u_ps[:])
            nc.sync.dma_start(
                out=u_d.ap().rearrange("c p e -> p c e"), in_=u_sb[:]
            )
            nc.sync.dma_start(out=d_d.ap(), in_=den_parts[:])
    nc.compile()
    return nc


def _run(nc: bass.Bass, in_maps, **kw):
    return run_bass_kernel_spmd(nc, in_maps, list(range(len(in_maps))), **kw)


def make_consts(W: np.ndarray) -> np.ndarray:
    """W.T as 8 stationary [128, 128] tiles: wt[p, (ec*2+kc)*128+i]."""
    wt = np.ascontiguousarray(W.T, np.float32).astype(BF)  # [E, K]
    out = np.zeros((128, 8 * 128), BF)
    for ec in range(4):
        for kc in range(2):
            j = ec * 2 + kc
            out[:, j * 128 : (j + 1) * 128] = wt[
                ec * 128 : (ec + 1) * 128, kc * 128 : (kc + 1) * 128
            ]
    return out


def make_in_maps(x: np.ndarray, W: np.ndarray):
    xf = np.asarray(x, np.float32).reshape(B, M, E)
    wt = make_consts(W)
    nblk = RPC // MBLK
    in_maps = []
    for i in range(NCORES):
        bi, si = divmod(i, CORES_PER_B)
        shard = xf[bi, si * RPC : (si + 1) * RPC]  # [rows, E]
        # x8[p, blk, j, e] = shard[blk*512 + j*128 + p, e]
        x8 = np.ascontiguousarray(
            shard.reshape(nblk, 4, 128, E).transpose(2, 0, 1, 3).astype(F8)
        )
        # xt[p, blk, ec, mm] = shard[blk*512 + mm, ec*128 + p]
        xt = np.ascontiguousarray(
            shard.astype(BF)
            .reshape(nblk, MBLK, 4, 128)
            .transpose(3, 0, 2, 1)
        )
        in_maps.append({"x8": x8, "xt": xt, "wt": wt})
    return in_maps


def combine(results) -> np.ndarray:
    """Sum per-core partials per batch, normalize, stack."""
    out = np.empty((B, K, E), np.float32)
    for bi in range(B):
        U = np.zeros((K, E), np.float64)
        den = np.zeros((K,), np.float64)
        for si in range(CORES_PER_B):
            r = results[bi * CORES_PER_B + si]
            U += r["u"].reshape(K, E).astype(np.float64)
            # d is [128, 2, nblk]: k = c*128 + p, sum the block partials
            den += r["d"].sum(axis=-1).T.reshape(K).astype(np.float64)
        out[bi] = (U / den[:, None]).astype(np.float32)
    return out


_NC_CACHE: dict[int, bass.Bass] = {}


def kernel(x: np.ndarray, W: np.ndarray, b_bias: np.ndarray) -> np.ndarray:
    # b_bias shifts every column of scores by a constant along the softmax
    # axis -> cancels in softmax; unused by construction.
    if RPC not in _NC_CACHE:
        _NC_CACHE[RPC] = build_nc(RPC)
    res = _run(_NC_CACHE[RPC], make_in_maps(np.asarray(x), np.asarray(W)))
    return combine(res.results)


# revision 5
# speedup vs baseline: 3.5262x; 3.5262x over previous
"""DifferentiableTokenSelection Trainium2 kernel (v3).

Math (reference):
    x: [b=2, t=64, n=1024, e=512] -> x_flat [b, m=65536, e]
    scores  = x_flat @ W.T + bias            [b, m, k=256]
    weights = softmax(scores / tau, axis=m)  (tau = 1.0)
    out     = einsum('bmk,bme->bke', weights, x_flat)   [b, 256, 512]

Key simplifications (exact, not approximations):
  * softmax over m is invariant to per-(b,k) constant shifts -> the bias
    cancels entirely; ignore b_bias.
  * scores ~ N(0,1), max |s| ~ 6 -> exp() without max-subtraction is safe
    in fp32. Single streaming pass: U[k,e] = sum_m exp(s[m,k]) x[m,e] and
    denom[k] = sum_m exp(s[m,k]) accumulate in PSUM; out = U / denom.
  * numerator and denominator use the SAME quantized weights, so weight
    quantization largely cancels in the ratio.

Layouts/dtypes (v3 changes vs v1 marked *):
  * mm1 (scores) in bf16: host pre-transposes x per 256-token pair
    (xt[p, pr, ec, mm]); x^T subtiles are the stationary operand, W.T
    chunks stream (256 cols each). bf16 128-col weight loads get the
    compiler's fast-weight-load (4 xbus) path and hide under streaming.
  * mm2 (pooling) in fp8e4m3 DoubleRow; PSUM accumulation fp32.
  * (*) denominators ride mm2 for free: the host appends a ones column
    to each 256-wide half of x (padded to 272 for DR's 16-alignment), so
    mm2 is 4 matmuls of N=257 per pair and den[k] is output column 256.
    This removes v1's 128 extra den matmuls and their DoubleRow
    LDWEIGHTS traffic (~27us of weight-port demand).
  * (*) mm2 for pair p is emitted LAG pairs behind mm1 so the in-order
    PE queue never waits on the scalar exp chain.
  * (*) per-pair DMAs (256 KB) instead of 16-subtile blocks: first
    matmul starts ~2us in instead of ~27us.
  * (*) all DRAM tensors are laid out exactly like their SBUF
    destination (>=1KB contiguous runs per partition) for DMA
    efficiency.

Sharding: batch x token-axis. core i handles batch i//4, m-rows
[16384*(i%4), 16384*(i%4+1)). Each core emits partial U and denom; the
host sums the 4 partials per batch and divides (gather/unshard step).
"""

import numpy as np
import ml_dtypes

import concourse.bacc as bacc
import concourse.bass as bass
import concourse.tile as tile
from concourse import mybir
from concourse.bass_utils import run_bass_kernel_spmd

B, T, NTOK, E, K = 2, 64, 1024, 512, 256
M = T * NTOK                 # 65536 tokens per batch
NCORES = 8
CORES_PER_B = NCORES // B    # 4
RPC = M // CORES_PER_B       # 16384 rows per core
PAIR = 256                   # tokens per mm subtile-pair
LAG = 2                      # mm2 trails mm1 by this many pairs
EH = 272                     # padded half-width: 256 x-cols + ones + pad

F32 = mybir.dt.float32
BF16 = mybir.dt.bfloat16
FP8 = mybir.dt.float8e4
EXP = mybir.ActivationFunctionType.Exp
BF = ml_dtypes.bfloat16
F8 = ml_dtypes.float8_e4m3
DR = mybir.MatmulPerfMode.DoubleRow


def build_nc(
    rows: int,
    xt_bufs: int = 4,
    xin_bufs: int = LAG + 3,
    wexp_bufs: int = LAG + 2,
    sc_bufs: int = 3,
) -> bass.Bass:
    """Emit the per-core bass program for `rows` m-rows."""
    npair = rows // PAIR
    assert rows % PAIR == 0

    nc = bacc.Bacc("TRN2", target_bir_lowering=False, debug=False)
    # mm1 stationary: xt[p, pr, ec, mm] = x[pr*256 + mm, ec*128 + p], bf16
    xt_d = nc.dram_tensor("xt", [128, npair, 4, PAIR], BF16, kind="ExternalInput")
    # mm2 moving: x8[p, pr, j, :] = [x[r,0:256], 1, 0*15, x[r,256:512], 1,
    # 0*15] with r = pr*256 + j*128 + p, fp8
    x8_d = nc.dram_tensor("x8", [128, npair, 2, 2 * EH], FP8, kind="ExternalInput")
    # mm1 moving: W.T chunks, consts[p, ec*256 + k] = W.T[ec*128 + p, k]
    c_d = nc.dram_tensor("consts", [128, 4 * K], BF16, kind="ExternalInput")
    # u[p, kc*2+eh, :] = [U[kc*128+p, eh*256 : eh*256+256], den-or-dup]
    u_d = nc.dram_tensor("u", [128, 4, 257], F32, kind="ExternalOutput")

    with tile.TileContext(nc) as tc:
        with (
            tc.tile_pool(name="const", bufs=1) as constp,
            tc.tile_pool(name="xt", bufs=xt_bufs) as xtp,
            tc.tile_pool(name="xin", bufs=xin_bufs) as xinp,
            tc.tile_pool(name="wexp", bufs=wexp_bufs) as wexpp,
            tc.tile_pool(name="outs", bufs=1) as outp,
            tc.tile_pool(name="ps_sc", bufs=sc_bufs, space="PSUM") as ps_sc,
            tc.tile_pool(name="ps_acc", bufs=1, space="PSUM") as ps_acc,
        ):
            consts = constp.tile([128, 4 * K], BF16)
            nc.sync.dma_start(out=consts[:], in_=c_d.ap())
            nexp_bias = constp.tile([128, 1], F32)
            nc.gpsimd.memset(nexp_bias[:], -2.7725887)  # -ln(16)

            u_ps = ps_acc.tile([128, 4, 512], F32)  # 4 banks, live all kernel

            def emit_front(pr):
                """loads + mm1 + exp for subtile-pair pr."""
                xtb = xtp.tile([128, 4, PAIR], BF16, tag="xtb")
                nc.sync.dma_start(out=xtb[:], in_=xt_d.ap()[:, pr])
                x8b = xinp.tile([128, 2, 2 * EH], FP8, tag="x8b")
                nc.gpsimd.dma_start(out=x8b[:], in_=x8_d.ap()[:, pr])

                # mm1: scores[m, k] for the subtile pair; x^T stationary,
                # W.T streaming. start=True clears the whole psum bank ->
                # only on the very first matmul of the pair.
                sc = ps_sc.tile([128, 2, K], F32, tag="sc")
                for jj in range(2):
                    for ec in range(4):
                        nc.tensor.matmul(
                            sc[:, jj, :],
                            xtb[:, ec, jj * 128 : (jj + 1) * 128],
                            consts[:, ec * K : (ec + 1) * K],
                            start=(ec == 0 and jj == 0),
                            stop=(ec == 3 and jj == 1),
                            skip_group_check=True,
                        )
                # exp(s - ln16) keeps the weights within fp8e4m3 range
                # (max ~240; raw exp(s) can reach ~270). The 1/16 scale
                # hits numerator and denominator alike -> exact cancel.
                wexp = wexpp.tile([128, 2, K], FP8, tag="wexp")
                nc.scalar.activation(wexp[:], sc[:], EXP, bias=nexp_bias[:])
                return x8b, wexp

            def emit_mm2(pr, x8b, wexp):
                first, last = pr == 0, pr == npair - 1
                for kc in range(2):
                    for eh in range(2):
                        nc.tensor.matmul(
                            u_ps[:, kc * 2 + eh, 0:257],
                            wexp[:, :, kc * 128 : (kc + 1) * 128],
                            x8b[:, :, eh * EH : eh * EH + 257],
                            start=first,
                            stop=last,
                            perf_mode=DR,
                        )

            pending = []
            for pr in range(npair):
                pending.append((pr, *emit_front(pr)))
                if pr >= LAG:
                    emit_mm2(*pending.pop(0))
            for item in pending:
                emit_mm2(*item)

            u_sb = outp.tile([128, 4, 257], F32)
            nc.vector.tensor_copy(u_sb[:], u_ps[:, :, 0:257])
            nc.sync.dma_start(out=u_d.ap(), in_=u_sb[:])
    nc.compile()
    return nc


def _run(nc: bass.Bass, in_maps, **kw):
    return run_bass_kernel_spmd(nc, in_maps, list(range(len(in_maps))), **kw)


def make_consts(W: np.ndarray) -> np.ndarray:
    """W.T as [c p] k chunks per partition, bf16."""
    consts = np.zeros((128, 4 * K), BF)
    wt = np.ascontiguousarray(W.T, np.float32).astype(BF)  # [E, K]
    for c in range(4):
        consts[:, c * K : (c + 1) * K] = wt[c * 128 : (c + 1) * 128, :]
    return consts


def make_in_maps(x: np.ndarray, W: np.ndarray):
    xf = np.asarray(x, np.float32).reshape(B, M, E)
    consts = make_consts(W)
    npair = RPC // PAIR
    in_maps = []
    for i in range(NCORES):
        bi, si = divmod(i, CORES_PER_B)
        shard = xf[bi, si * RPC : (si + 1) * RPC]  # [rows, E]
        # x8 with ones columns: per row [x[0:256], 1, 0*15, x[256:512], 1, 0*15]
        rows8 = np.zeros((RPC, 2 * EH), F8)
        rows8[:, 0:256] = shard[:, 0:256].astype(F8)
        rows8[:, 256] = 1.0
        rows8[:, EH : EH + 256] = shard[:, 256:512].astype(F8)
        rows8[:, EH + 256] = 1.0
        # x8[p, pr, j, c] = rows8[pr*256 + j*128 + p, c]
        x8 = np.ascontiguousarray(
            rows8.reshape(npair, 2, 128, 2 * EH).transpose(2, 0, 1, 3)
        )
        # xt[p, pr, ec, mm] = shard[pr*256 + mm, ec*128 + p]
        xt = np.ascontiguousarray(
            shard.astype(BF).reshape(npair, PAIR, 4, 128).transpose(3, 0, 2, 1)
        )
        in_maps.append({"x8": x8, "xt": xt, "consts": consts})
    return in_maps


def combine(results) -> np.ndarray:
    """Sum per-core partials per batch, normalize, stack."""
    out = np.empty((B, K, E), np.float32)
    for bi in range(B):
        U = np.zeros((K, E), np.float64)
        den = np.zeros((K,), np.float64)
        for si in range(CORES_PER_B):
            u = results[bi * CORES_PER_B + si]["u"].astype(np.float64)
            # u[p, kc*2+eh, 0:256] = U-part[kc*128+p, eh*256:+256]
            for kc in range(2):
                for eh in range(2):
                    U[kc * 128 : (kc + 1) * 128, eh * 256 : (eh + 1) * 256] += u[
                        :, kc * 2 + eh, 0:256
                    ]
                den[kc * 128 : (kc + 1) * 128] += u[:, kc * 2, 256]
        out[bi] = (U / den[:, None]).astype(np.float32)
    return out


_NC_CACHE: dict[int, bass.Bass] = {}


def kernel(x: np.ndarray, W: np.ndarray, b_bias: np.ndarray) -> np.ndarray:
    # b_bias shifts every column of scores by a constant along the softmax
    # axis -> cancels in softmax; unused by construction.
    if RPC not in _NC_CACHE:
        _NC_CACHE[RPC] = build_nc(RPC)
    res = _run(_NC_CACHE[RPC], make_in_maps(np.asarray(x), np.asarray(W)))
    return combine(res.results)


# revision 11
# speedup vs baseline: 3.8096x; 1.0804x over previous
"""DifferentiableTokenSelection Trainium2 kernel (v3).

Math (reference):
    x: [b=2, t=64, n=1024, e=512] -> x_flat [b, m=65536, e]
    scores  = x_flat @ W.T + bias            [b, m, k=256]
    weights = softmax(scores / tau, axis=m)  (tau = 1.0)
    out     = einsum('bmk,bme->bke', weights, x_flat)   [b, 256, 512]

Key simplifications (exact, not approximations):
  * softmax over m is invariant to per-(b,k) constant shifts -> the bias
    cancels entirely; ignore b_bias.
  * scores ~ N(0,1), max |s| ~ 6 -> exp() without max-subtraction is safe
    in fp32. Single streaming pass: U[k,e] = sum_m exp(s[m,k]) x[m,e] and
    denom[k] = sum_m exp(s[m,k]) accumulate in PSUM; out = U / denom.
  * numerator and denominator use the SAME quantized weights, so weight
    quantization largely cancels in the ratio.

Layouts/dtypes (v3 changes vs v1 marked *):
  * mm1 (scores) in bf16: host pre-transposes x per 256-token pair
    (xt[p, pr, ec, mm]); x^T subtiles are the stationary operand, W.T
    chunks stream (256 cols each). bf16 128-col weight loads get the
    compiler's fast-weight-load (4 xbus) path and hide under streaming.
  * mm2 (pooling) in fp8e4m3 DoubleRow; PSUM accumulation fp32.
  * (*) denominators ride mm2 for free: the host appends a ones column
    to each 256-wide half of x (padded to 272 for DR's 16-alignment), so
    mm2 is 4 matmuls of N=257 per pair and den[k] is output column 256.
    This removes v1's 128 extra den matmuls and their DoubleRow
    LDWEIGHTS traffic (~27us of weight-port demand).
  * (*) mm2 for pair p is emitted LAG pairs behind mm1 so the in-order
    PE queue never waits on the scalar exp chain.
  * (*) per-pair DMAs (256 KB) instead of 16-subtile blocks: first
    matmul starts ~2us in instead of ~27us.
  * (*) all DRAM tensors are laid out exactly like their SBUF
    destination (>=1KB contiguous runs per partition) for DMA
    efficiency.

Sharding: batch x token-axis. core i handles batch i//4, m-rows
[16384*(i%4), 16384*(i%4+1)). Each core emits partial U and denom; the
host sums the 4 partials per batch and divides (gather/unshard step).
"""

import numpy as np
import ml_dtypes

import concourse.bacc as bacc
import concourse.bass as bass
import concourse.tile as tile
from concourse import mybir
from concourse.bass_utils import run_bass_kernel_spmd

B, T, NTOK, E, K = 2, 64, 1024, 512, 256
M = T * NTOK                 # 65536 tokens per batch
NCORES = 8
CORES_PER_B = NCORES // B    # 4
RPC = M // CORES_PER_B       # 16384 rows per core
PAIR = 256                   # tokens per mm subtile-pair
LAG = 2                      # mm2 trails mm1 by this many pairs
EH = 272                     # padded half-width: 256 x-cols + ones + pad

F32 = mybir.dt.float32
BF16 = mybir.dt.bfloat16
FP8 = mybir.dt.float8e4
FP8E3 = mybir.dt.float8e3
EXP = mybir.ActivationFunctionType.Exp
BF = ml_dtypes.bfloat16
F8 = ml_dtypes.float8_e4m3
E3 = ml_dtypes.float8_e3m4
DR = mybir.MatmulPerfMode.DoubleRow

# x^T (the mm1 stationary operand) in fp8e3m4: 4 mantissa bits keep the
# score noise negligible (sim: rel L2 6.2e-3 vs 5.9e-3 for bf16) while
# halving the dominant DMA stream. Non-DoubleRow fp8 runs at bf16 speed.
XT_E3M4 = True
XT_DT = FP8E3 if XT_E3M4 else BF16
XT_NP = E3 if XT_E3M4 else BF


def build_nc(
    rows: int,
    xt_bufs: int = 8,
    xin_bufs: int = 8,
    wexp_bufs: int = LAG + 2,
    sc_bufs: int = 4,
) -> bass.Bass:
    """Emit the per-core bass program for `rows` m-rows."""
    npair = rows // PAIR
    assert rows % PAIR == 0

    nc = bacc.Bacc("TRN2", target_bir_lowering=False, debug=False)
    # mm1 stationary: xt[p, pr, ec, mm] = x[pr*256 + mm, ec*128 + p]
    xt_d = nc.dram_tensor("xt", [128, npair, 4, PAIR], XT_DT, kind="ExternalInput")
    # mm2 moving: x8[p, pr, j, :] = [x[r,0:256], 1, 0*15, x[r,256:512], 1,
    # 0*15] with r = pr*256 + j*128 + p, fp8
    x8_d = nc.dram_tensor("x8", [128, npair, 2, 2 * EH], FP8, kind="ExternalInput")
    # mm1 moving: W.T chunks, consts[p, ec*256 + k] = W.T[ec*128 + p, k]
    c_d = nc.dram_tensor("consts", [128, 4 * K], BF16, kind="ExternalInput")
    # u[p, kc*2+eh, :] = [U[kc*128+p, eh*256 : eh*256+256], den-or-dup]
    u_d = nc.dram_tensor("u", [128, 4, 257], F32, kind="ExternalOutput")

    with tile.TileContext(nc) as tc:
        with (
            tc.tile_pool(name="const", bufs=1) as constp,
            tc.tile_pool(name="xt", bufs=xt_bufs) as xtp,
            tc.tile_pool(name="xin", bufs=xin_bufs) as xinp,
            tc.tile_pool(name="wexp", bufs=wexp_bufs) as wexpp,
            tc.tile_pool(name="ps_sc", bufs=sc_bufs, space="PSUM") as ps_sc,
            tc.tile_pool(name="ps_acc", bufs=1, space="PSUM") as ps_acc,
        ):
            # split the consts load per ec-chunk so the first matmul only
            # waits on 64KB, not the whole table
            consts = constp.tile([128, 4 * K], BF16)
            for ec in range(4):
                nc.sync.dma_start(
                    out=consts[:, ec * K : (ec + 1) * K],
                    in_=c_d.ap()[:, ec * K : (ec + 1) * K],
                )
            nexp_bias = constp.tile([128, 1], F32)
            nc.gpsimd.memset(nexp_bias[:], -2.7725887)  # -ln(16)

            u_ps = ps_acc.tile([128, 4, 512], F32)  # 4 banks, live all kernel

            def emit_front(pr):
                """loads + mm1 + exp for subtile-pair pr."""
                xtb = xtp.tile([128, 4, PAIR], XT_DT, tag="xtb")
                if pr == 0:  # fine-grained first load: PE starts sooner
                    for ec in range(4):
                        nc.sync.dma_start(
                            out=xtb[:, ec], in_=xt_d.ap()[:, pr, ec]
                        )
                else:
                    nc.sync.dma_start(out=xtb[:], in_=xt_d.ap()[:, pr])
                x8b = xinp.tile([128, 2, 2 * EH], FP8, tag="x8b")
                nc.gpsimd.dma_start(out=x8b[:], in_=x8_d.ap()[:, pr])

                # mm1: scores[m, k] for the subtile pair; x^T stationary,
                # W.T streaming. start=True clears the whole psum bank ->
                # only on the very first matmul of the pair.
                sc = ps_sc.tile([128, 2, K], F32, tag="sc")
                for jj in range(2):
                    for ec in range(4):
                        nc.tensor.matmul(
                            sc[:, jj, :],
                            xtb[:, ec, jj * 128 : (jj + 1) * 128],
                            consts[:, ec * K : (ec + 1) * K],
                            start=(ec == 0 and jj == 0),
                            stop=(ec == 3 and jj == 1),
                            skip_group_check=True,
                        )
                # exp(s - ln16) keeps the weights within fp8e4m3 range
                # (max ~240; raw exp(s) can reach ~270). The 1/16 scale
                # hits numerator and denominator alike -> exact cancel.
                wexp = wexpp.tile([128, 2, K], FP8, tag="wexp")
                nc.scalar.activation(wexp[:], sc[:], EXP, bias=nexp_bias[:])
                return x8b, wexp

            def emit_mm2(pr, x8b, wexp):
                first, last = pr == 0, pr == npair - 1
                for kc in range(2):
                    for eh in range(2):
                        nc.tensor.matmul(
                            u_ps[:, kc * 2 + eh, 0:257],
                            wexp[:, :, kc * 128 : (kc + 1) * 128],
                            x8b[:, :, eh * EH : eh * EH + 257],
                            start=first,
                            stop=last,
                            perf_mode=DR,
                        )

            pending = []
            for pr in range(npair):
                pending.append((pr, *emit_front(pr)))
                if pr >= LAG:
                    emit_mm2(*pending.pop(0))
            for item in pending:
                emit_mm2(*item)

            u_sb = constp.tile([128, 4, 257], F32)
            nc.vector.tensor_copy(u_sb[:], u_ps[:, :, 0:257])
            nc.sync.dma_start(out=u_d.ap(), in_=u_sb[:])
    nc.compile()
    return nc


def _run(nc: bass.Bass, in_maps, **kw):
    return run_bass_kernel_spmd(nc, in_maps, list(range(len(in_maps))), **kw)


def make_consts(W: np.ndarray) -> np.ndarray:
    """W.T as [c p] k chunks per partition, bf16."""
    consts = np.zeros((128, 4 * K), BF)
    wt = np.ascontiguousarray(W.T, np.float32).astype(BF)  # [E, K]
    for c in range(4):
        consts[:, c * K : (c + 1) * K] = wt[c * 128 : (c + 1) * 128, :]
    return consts


def make_in_maps(x: np.ndarray, W: np.ndarray):
    xf = np.asarray(x, np.float32).reshape(B, M, E)
    consts = make_consts(W)
    npair = RPC // PAIR
    in_maps = []
    for i in range(NCORES):
        bi, si = divmod(i, CORES_PER_B)
        shard = xf[bi, si * RPC : (si + 1) * RPC]  # [rows, E]
        # x8 with ones columns: per row [x[0:256], 1, 0*15, x[256:512], 1, 0*15]
        rows8 = np.zeros((RPC, 2 * EH), F8)
        rows8[:, 0:256] = shard[:, 0:256].astype(F8)
        rows8[:, 256] = 1.0
        rows8[:, EH : EH + 256] = shard[:, 256:512].astype(F8)
        rows8[:, EH + 256] = 1.0
        # x8[p, pr, j, c] = rows8[pr*256 + j*128 + p, c]
        x8 = np.ascontiguousarray(
            rows8.reshape(npair, 2, 128, 2 * EH).transpose(2, 0, 1, 3)
        )
        # xt[p, pr, ec, mm] = shard[pr*256 + mm, ec*128 + p]
        xt = np.ascontiguousarray(
            shard.astype(XT_NP).reshape(npair, PAIR, 4, 128).transpose(3, 0, 2, 1)
        )
        in_maps.append({"x8": x8, "xt": xt, "consts": consts})
    return in_maps


def combine(results) -> np.ndarray:
    """Sum per-core partials per batch, normalize, stack."""
    out = np.empty((B, K, E), np.float32)
    for bi in range(B):
        U = np.zeros((K, E), np.float64)
        den = np.zeros((K,), np.float64)
        for si in range(CORES_PER_B):
            u = results[bi * CORES_PER_B + si]["u"].astype(np.float64)
            # u[p, kc*2+eh, 0:256] = U-part[kc*128+p, eh*256:+256]
            for kc in range(2):
                for eh in range(2):
                    U[kc * 128 : (kc + 1) * 128, eh * 256 : (eh + 1) * 256] += u[
                        :, kc * 2 + eh, 0:256
                    ]
                den[kc * 128 : (kc + 1) * 128] += u[:, kc * 2, 256]
        out[bi] = (U / den[:, None]).astype(np.float32)
    return out


_NC_CACHE: dict[int, bass.Bass] = {}


def kernel(x: np.ndarray, W: np.ndarray, b_bias: np.ndarray) -> np.ndarray:
    # b_bias shifts every column of scores by a constant along the softmax
    # axis -> cancels in softmax; unused by construction.
    if RPC not in _NC_CACHE:
        _NC_CACHE[RPC] = build_nc(RPC)
    res = _run(_NC_CACHE[RPC], make_in_maps(np.asarray(x), np.asarray(W)))
    return combine(res.results)


# revision 14
# speedup vs baseline: 3.8625x; 1.0139x over previous
"""DifferentiableTokenSelection Trainium2 kernel (v3).

Math (reference):
    x: [b=2, t=64, n=1024, e=512] -> x_flat [b, m=65536, e]
    scores  = x_flat @ W.T + bias            [b, m, k=256]
    weights = softmax(scores / tau, axis=m)  (tau = 1.0)
    out     = einsum('bmk,bme->bke', weights, x_flat)   [b, 256, 512]

Key simplifications (exact, not approximations):
  * softmax over m is invariant to per-(b,k) constant shifts -> the bias
    cancels entirely; ignore b_bias.
  * scores ~ N(0,1), max |s| ~ 6 -> exp() without max-subtraction is safe
    in fp32. Single streaming pass: U[k,e] = sum_m exp(s[m,k]) x[m,e] and
    denom[k] = sum_m exp(s[m,k]) accumulate in PSUM; out = U / denom.
  * numerator and denominator use the SAME quantized weights, so weight
    quantization largely cancels in the ratio.

Layouts/dtypes (v3 changes vs v1 marked *):
  * mm1 (scores) in bf16: host pre-transposes x per 256-token pair
    (xt[p, pr, ec, mm]); x^T subtiles are the stationary operand, W.T
    chunks stream (256 cols each). bf16 128-col weight loads get the
    compiler's fast-weight-load (4 xbus) path and hide under streaming.
  * mm2 (pooling) in fp8e4m3 DoubleRow; PSUM accumulation fp32.
  * (*) denominators ride mm2 for free: the host appends a ones column
    to each 256-wide half of x (padded to 272 for DR's 16-alignment), so
    mm2 is 4 matmuls of N=257 per pair and den[k] is output column 256.
    This removes v1's 128 extra den matmuls and their DoubleRow
    LDWEIGHTS traffic (~27us of weight-port demand).
  * (*) mm2 for pair p is emitted LAG pairs behind mm1 so the in-order
    PE queue never waits on the scalar exp chain.
  * (*) per-pair DMAs (256 KB) instead of 16-subtile blocks: first
    matmul starts ~2us in instead of ~27us.
  * (*) all DRAM tensors are laid out exactly like their SBUF
    destination (>=1KB contiguous runs per partition) for DMA
    efficiency.

Sharding: batch x token-axis. core i handles batch i//4, m-rows
[16384*(i%4), 16384*(i%4+1)). Each core emits partial U and denom; the
host sums the 4 partials per batch and divides (gather/unshard step).
"""

import numpy as np
import ml_dtypes

import concourse.bacc as bacc
import concourse.bass as bass
import concourse.tile as tile
from concourse import mybir
from concourse.bass_utils import run_bass_kernel_spmd

B, T, NTOK, E, K = 2, 64, 1024, 512, 256
M = T * NTOK                 # 65536 tokens per batch
NCORES = 8
CORES_PER_B = NCORES // B    # 4
RPC = M // CORES_PER_B       # 16384 rows per core
PAIR = 256                   # tokens per mm subtile-pair
LAG = 2                      # mm2 trails mm1 by this many pairs
EH = 272                     # padded half-width: 256 x-cols + ones + pad

F32 = mybir.dt.float32
BF16 = mybir.dt.bfloat16
FP8 = mybir.dt.float8e4
FP8E3 = mybir.dt.float8e3
EXP = mybir.ActivationFunctionType.Exp
BF = ml_dtypes.bfloat16
F8 = ml_dtypes.float8_e4m3
E3 = ml_dtypes.float8_e3m4
DR = mybir.MatmulPerfMode.DoubleRow

# x^T (the mm1 stationary operand) in fp8e3m4: 4 mantissa bits keep the
# score noise negligible (sim: rel L2 6.2e-3 vs 5.9e-3 for bf16) while
# halving the dominant DMA stream. Non-DoubleRow fp8 runs at bf16 speed.
XT_E3M4 = True
XT_DT = FP8E3 if XT_E3M4 else BF16
XT_NP = E3 if XT_E3M4 else BF


def build_nc(
    rows: int,
    xt_bufs: int = 3,
    xin_bufs: int = 3,
    wexp_bufs: int = LAG + 2,
    sc_bufs: int = 4,
) -> bass.Bass:
    """Emit the per-core bass program for `rows` m-rows."""
    npair = rows // PAIR
    assert rows % PAIR == 0

    nc = bacc.Bacc("TRN2", target_bir_lowering=False, debug=False)
    # mm1 stationary: xt[p, pr, ec, mm] = x[pr*256 + mm, ec*128 + p]
    xt_d = nc.dram_tensor("xt", [128, npair, 4, PAIR], XT_DT, kind="ExternalInput")
    # mm2 moving: x8[p, pr, j, :] = [x[r,0:256], 1, 0*15, x[r,256:512], 1,
    # 0*15] with r = pr*256 + j*128 + p, fp8
    x8_d = nc.dram_tensor("x8", [128, npair, 2, 2 * EH], FP8, kind="ExternalInput")
    # mm1 moving: W.T chunks, consts[p, ec*256 + k] = W.T[ec*128 + p, k]
    c_d = nc.dram_tensor("consts", [128, 4 * K], BF16, kind="ExternalInput")
    # u[p, kc*2+eh, :] = [U[kc*128+p, eh*256 : eh*256+256], den-or-dup]
    u_d = nc.dram_tensor("u", [128, 4, 257], F32, kind="ExternalOutput")

    with tile.TileContext(nc) as tc:
        with (
            tc.tile_pool(name="const", bufs=1) as constp,
            tc.tile_pool(name="xt", bufs=xt_bufs) as xtp,
            tc.tile_pool(name="xin", bufs=xin_bufs) as xinp,
            tc.tile_pool(name="wexp", bufs=wexp_bufs) as wexpp,
            tc.tile_pool(name="ps_sc", bufs=sc_bufs, space="PSUM") as ps_sc,
            tc.tile_pool(name="ps_acc", bufs=1, space="PSUM") as ps_acc,
        ):
            consts = constp.tile([128, 4 * K], BF16)
            nc.sync.dma_start(out=consts[:], in_=c_d.ap())
            nexp_bias = constp.tile([128, 1], F32)
            nc.gpsimd.memset(nexp_bias[:], -2.7725887)  # -ln(16)

            u_ps = ps_acc.tile([128, 4, 512], F32)  # 4 banks, live all kernel

            # Each dma_start costs ~0.6us of serial descriptor-gen (DIRECT2D)
            # on its dispatch queue, so batch the streaming loads 4 pairs at
            # a time. The first xt batch is split in two so the PE's first
            # matmul only waits on a 2-pair transfer.
            BATCH = 4
            xt_tiles = {}
            x8_tiles = {}

            def emit_front(pr):
                """loads + mm1 + exp for subtile-pair pr."""
                if pr % BATCH == 0:
                    xtb4 = xtp.tile([128, BATCH, 4, PAIR], XT_DT, tag="xtb")
                    if pr == 0:
                        h = BATCH // 2
                        nc.sync.dma_start(
                            out=xtb4[:, :h], in_=xt_d.ap()[:, :h]
                        )
                        nc.sync.dma_start(
                            out=xtb4[:, h:BATCH], in_=xt_d.ap()[:, h:BATCH]
                        )
                    else:
                        nc.sync.dma_start(
                            out=xtb4[:], in_=xt_d.ap()[:, pr : pr + BATCH]
                        )
                    x8b4 = xinp.tile([128, BATCH, 2, 2 * EH], FP8, tag="x8b")
                    nc.gpsimd.dma_start(
                        out=x8b4[:], in_=x8_d.ap()[:, pr : pr + BATCH]
                    )
                    for q in range(BATCH):
                        xt_tiles[pr + q] = xtb4[:, q]
                        x8_tiles[pr + q] = x8b4[:, q]
                xtb = xt_tiles.pop(pr)
                x8b = x8_tiles.pop(pr)

                # mm1: scores[m, k] for the subtile pair; x^T stationary,
                # W.T streaming. start=True clears the whole psum bank ->
                # only on the very first matmul of the pair.
                sc = ps_sc.tile([128, 2, K], F32, tag="sc")
                for jj in range(2):
                    for ec in range(4):
                        nc.tensor.matmul(
                            sc[:, jj, :],
                            xtb[:, ec, jj * 128 : (jj + 1) * 128],
                            consts[:, ec * K : (ec + 1) * K],
                            start=(ec == 0 and jj == 0),
                            stop=(ec == 3 and jj == 1),
                            skip_group_check=True,
                        )
                # exp(s - ln16) keeps the weights within fp8e4m3 range
                # (max ~240; raw exp(s) can reach ~270). The 1/16 scale
                # hits numerator and denominator alike -> exact cancel.
                wexp = wexpp.tile([128, 2, K], FP8, tag="wexp")
                nc.scalar.activation(wexp[:], sc[:], EXP, bias=nexp_bias[:])
                return x8b, wexp

            def emit_mm2(pr, x8b, wexp):
                first, last = pr == 0, pr == npair - 1
                for kc in range(2):
                    for eh in range(2):
                        nc.tensor.matmul(
                            u_ps[:, kc * 2 + eh, 0:257],
                            wexp[:, :, kc * 128 : (kc + 1) * 128],
                            x8b[:, :, eh * EH : eh * EH + 257],
                            start=first,
                            stop=last,
                            perf_mode=DR,
                        )

            pending = []
            for pr in range(npair):
                pending.append((pr, *emit_front(pr)))
                if pr >= LAG:
                    emit_mm2(*pending.pop(0))
            for item in pending:
                emit_mm2(*item)

            u_sb = constp.tile([128, 4, 257], F32)
            nc.vector.tensor_copy(u_sb[:], u_ps[:, :, 0:257])
            nc.sync.dma_start(out=u_d.ap(), in_=u_sb[:])
    nc.compile()
    return nc


def _run(nc: bass.Bass, in_maps, **kw):
    return run_bass_kernel_spmd(nc, in_maps, list(range(len(in_maps))), **kw)


def make_consts(W: np.ndarray) -> np.ndarray:
    """W.T as [c p] k chunks per partition, bf16."""
    consts = np.zeros((128, 4 * K), BF)
    wt = np.ascontiguousarray(W.T, np.float32).astype(BF)  # [E, K]
    for c in range(4):
        consts[:, c * K : (c + 1) * K] = wt[c * 128 : (c + 1) * 128, :]
    return consts


def make_in_maps(x: np.ndarray, W: np.ndarray):
    xf = np.asarray(x, np.float32).reshape(B, M, E)
    consts = make_consts(W)
    npair = RPC // PAIR
    in_maps = []
    for i in range(NCORES):
        bi, si = divmod(i, CORES_PER_B)
        shard = xf[bi, si * RPC : (si + 1) * RPC]  # [rows, E]
        # x8 with ones columns: per row [x[0:256], 1, 0*15, x[256:512], 1, 0*15]
        rows8 = np.zeros((RPC, 2 * EH), F8)
        rows8[:, 0:256] = shard[:, 0:256].astype(F8)
        rows8[:, 256] = 1.0
        rows8[:, EH : EH + 256] = shard[:, 256:512].astype(F8)
        rows8[:, EH + 256] = 1.0
        # x8[p, pr, j, c] = rows8[pr*256 + j*128 + p, c]
        x8 = np.ascontiguousarray(
            rows8.reshape(npair, 2, 128, 2 * EH).transpose(2, 0, 1, 3)
        )
        # xt[p, pr, ec, mm] = shard[pr*256 + mm, ec*128 + p]
        xt = np.ascontiguousarray(
            shard.astype(XT_NP).reshape(npair, PAIR, 4, 128).transpose(3, 0, 2, 1)
        )
        in_maps.append({"x8": x8, "xt": xt, "consts": consts})
    return in_maps


def combine(results) -> np.ndarray:
    """Sum per-core partials per batch, normalize, stack."""
    out = np.empty((B, K, E), np.float32)
    for bi in range(B):
        U = np.zeros((K, E), np.float64)
        den = np.zeros((K,), np.float64)
        for si in range(CORES_PER_B):
            u = results[bi * CORES_PER_B + si]["u"].astype(np.float64)
            # u[p, kc*2+eh, 0:256] = U-part[kc*128+p, eh*256:+256]
            for kc in range(2):
                for eh in range(2):
                    U[kc * 128 : (kc + 1) * 128, eh * 256 : (eh + 1) * 256] += u[
                        :, kc * 2 + eh, 0:256
                    ]
                den[kc * 128 : (kc + 1) * 128] += u[:, kc * 2, 256]
        out[bi] = (U / den[:, None]).astype(np.float32)
    return out


_NC_CACHE: dict[int, bass.Bass] = {}


def kernel(x: np.ndarray, W: np.ndarray, b_bias: np.ndarray) -> np.ndarray:
    # b_bias shifts every column of scores by a constant along the softmax
    # axis -> cancels in softmax; unused by construction.
    if RPC not in _NC_CACHE:
        _NC_CACHE[RPC] = build_nc(RPC)
    res = _run(_NC_CACHE[RPC], make_in_maps(np.asarray(x), np.asarray(W)))
    return combine(res.results)
